# revision 1
# baseline (speedup 1.0000x reference)
"""CombinedDynamicMarginLoss on 8 trn2 NeuronCores.

Strategy: data-parallel over the batch dim N=1024 -> 128 rows per core
(one full SBUF partition tile), each core sees all C=93431 classes so
every per-row reduction is core-local (no collectives).

Device per core (streaming, single pass over the 47.8MB shard):
  - out = 64 * x           (full [128, C] output, ACT engine)
  - rowmax = max_j g(x_ij) (g(x) = x * (x <= 0.4), DVE)
Host glue (1024 rows, negligible):
  - cos_y gather, exclusion of the label column from the max,
    arccos/cos margin math, scatter of final_phi*64 into the output.

The device max includes the label column j=y with the filter applied
(g(cos_y)); since all g values are >= 0 and C is large,
max_other = rowmax exactly whenever g(cos_y) < rowmax. The rare
ambiguous rows (g(cos_y) == rowmax) are recomputed exactly on host.
"""

import numpy as np

import concourse.bacc as bacc
import concourse.mybir as mybir
import concourse.tile as tile
from concourse.bass_utils import run_bass_kernel_spmd

N, C = 1024, 93431
NCORES = 8
R = N // NCORES  # 128 rows per core

S = 64.0
M1 = 1.0
M2 = 0.5
M3 = 0.0
ALPHA = 0.1
THRESH = 0.4
NEG_BIG = -1.0e9

T = 4096                      # column tile buffer width
# Variable tile widths: a small first tile starts the store stream early,
# a small last tile minimizes the compute-drain after the final load.
WIDTHS = [512] + [4096] * 22 + [2295] + [512]
assert sum(WIDTHS) == C and max(WIDTHS) == T
NT = len(WIDTHS)              # 25

_CACHE: dict = {}
LAST_RESULT = None            # BassKernelResults of the last run (for test.py)
RUN_KWARGS: dict = {}         # test.py can set {"trace": True}


def _build():
    f32 = mybir.dt.float32
    # Bacc (not raw Bass): its compile pass splits multi-wait sync onto
    # separate event-semaphore instructions — DMACopy only encodes 1 wait.
    nc = bacc.Bacc(None, enable_partition_id=False)
    x = nc.declare_dram_parameter("x", [R, C], f32, isOutput=False)
    y = nc.declare_dram_parameter("y", [R, C], f32, isOutput=True)
    mx = nc.declare_dram_parameter("mx", [R, NT], f32, isOutput=True)

    # 0.4 * 64 is exact in fp32 (power-of-two scale), so filtering the
    # scaled tensor (yt <= 25.6) * yt equals 64 * g(x) bit-exactly.
    thresh_s = float(np.float32(THRESH) * np.float32(S))

    # Loads on the sync HWDGE ring, stores on the scalar engine's HWDGE
    # ring (same-engine ordering after the mul that produced the data).
    with tile.TileContext(nc) as tc:
        with (
            tc.tile_pool(name="xin", bufs=4) as xpool,
            tc.tile_pool(name="yout", bufs=4) as ypool,
            tc.tile_pool(name="gtmp", bufs=2) as gpool,
            tc.tile_pool(name="stat", bufs=1) as statpool,
        ):
            maxbuf = statpool.tile([R, NT], f32)
            col = 0
            for t, w in enumerate(WIDTHS):
                xt = xpool.tile([R, T], f32, tag="xt")
                nc.sync.dma_start(out=xt[:, :w], in_=x[:, col : col + w])

                yt = ypool.tile([R, T], f32, tag="yt")
                nc.scalar.mul(yt[:, :w], xt[:, :w], S)
                nc.scalar.dma_start(out=y[:, col : col + w], in_=yt[:, :w])

                # g64 = (yt <= 25.6) * yt == 64 * g(x), one DVE op
                g = gpool.tile([R, T], f32, tag="g")
                nc.vector.scalar_tensor_tensor(
                    out=g[:, :w],
                    in0=yt[:, :w],
                    scalar=thresh_s,
                    in1=yt[:, :w],
                    op0=mybir.AluOpType.is_le,
                    op1=mybir.AluOpType.mult,
                )
                nc.vector.tensor_reduce(
                    out=maxbuf[:, t : t + 1],
                    in_=g[:, :w],
                    axis=mybir.AxisListType.X,
                    op=mybir.AluOpType.max,
                )
                col += w

            # ship the per-tile maxima; the final 23-column max runs on host
            nc.scalar.dma_start(out=mx[:], in_=maxbuf[:])
    # run_bass_via_pjrt serializes the module at jit-lowering time without
    # finalizing; Bacc's register allocation happens in finalize().
    nc.finalize()
    return nc


def _get_nc():
    if "nc" not in _CACHE:
        _CACHE["nc"] = _build()
    return _CACHE["nc"]


def kernel(logits, labels):
    global LAST_RESULT
    logits = np.ascontiguousarray(np.asarray(logits, dtype=np.float32))
    labels = np.asarray(labels).astype(np.int64)
    assert logits.shape == (N, C)

    nc = _get_nc()
    in_maps = [{"x": logits[k * R : (k + 1) * R]} for k in range(NCORES)]
    res = run_bass_kernel_spmd(nc, in_maps, list(range(NCORES)), **RUN_KWARGS)
    LAST_RESULT = res

    out = np.concatenate([res.results[k]["y"] for k in range(NCORES)], axis=0)
    M64 = np.concatenate([res.results[k]["mx"] for k in range(NCORES)], axis=0).max(axis=1)
    M = (M64 * np.float32(1.0 / S)).astype(np.float32)  # exact (power of two)

    # ---- host glue: per-row scalars (N=1024) ----
    valid = labels != -1
    lab = np.where(valid, labels, 0)
    rows = np.arange(N)
    cos_y = logits[rows, lab]                                   # f32
    g_cos = np.where(cos_y <= THRESH, cos_y, 0.0).astype(np.float32)

    max_other = M.copy()
    # ambiguous: the device max may have been achieved at the label column
    amb = np.nonzero((g_cos >= M) & valid)[0]
    for i in amb:
        g = np.where(logits[i] <= THRESH, logits[i], 0.0).astype(np.float32)
        g[lab[i]] = NEG_BIG
        max_other[i] = g.max()

    h = (np.float32(1.0) - (cos_y - max_other)).astype(np.float32)
    m_i = (np.float32(M2) + np.float32(ALPHA) * h).astype(np.float32)
    theta = np.arccos(np.clip(cos_y, -1.0, 1.0)).astype(np.float32)
    phi = (np.cos(np.float32(M1) * theta + m_i) - np.float32(M3)).astype(np.float32)
    final_phi = np.where(phi < cos_y, phi, cos_y).astype(np.float32)

    out[rows[valid], lab[valid]] = final_phi[valid] * np.float32(S)
    return out



# revision 2
# speedup vs baseline: 1.2011x; 1.2011x over previous
"""CombinedDynamicMarginLoss on 8 trn2 NeuronCores.

Strategy: data-parallel over the batch dim N=1024 -> 128 rows per core
(one full SBUF partition tile), each core sees all C=93431 classes so
every per-row reduction is core-local (no collectives).

The kernel is pure streaming (out = 64*x plus a per-row masked max),
so it is HBM-bandwidth-bound. The f32 version runs at ~97% of the
358 GB/s per-core HBM roofline; the only remaining lever is traffic.
Device I/O therefore uses bfloat16 (tolerance is 2e-2 rel; bf16
round-trip is <=0.4% rel), halving HBM bytes: read 23.9MB + write
23.9MB per core instead of 47.8+47.8.

Device per core (streaming, single pass over the 23.9MB bf16 shard):
  - out = 64 * xb          (full [128, C] bf16 output, ACT engine;
                            exact: power-of-two scale)
  - rowmax = max_j g(xb_ij) (g(v) = v * (64v <= 25.6), DVE; all values
                            are exact bf16*64 grid points)
Host glue (1024 rows, negligible):
  - f32<->bf16 conversion (sharding glue), cos_y gather, exclusion of
    the label column from the max, arccos/cos margin math, scatter of
    final_phi*64 into the output.

Precision guard: max_other from the bf16-domain max differs from the
f32 reference max by up to ~2e-3 (one bf16 ulp at 0.4, plus filter
boundary flips). That shifts phi by <= 2e-4, which only matters for
rows where |phi| is small (rel-err denominator). Rows with
|phi_est| < 0.1, and rows where the device max may be dominated by
the label column (exact bf16-domain comparison), are recomputed
exactly on host from the f32 logits (a few % of 1024 rows).
"""

import numpy as np
import ml_dtypes

import concourse.bacc as bacc
import concourse.mybir as mybir
import concourse.tile as tile
from concourse.bass_utils import run_bass_kernel_spmd

BF16 = ml_dtypes.bfloat16

N, C = 1024, 93431
NCORES = 8
R = N // NCORES  # 128 rows per core

S = 64.0
M1 = 1.0
M2 = 0.5
M3 = 0.0
ALPHA = 0.1
THRESH = 0.4
NEG_BIG = -1.0e9

T = 8192                      # column tile buffer width (16KB/partition bf16)
# Variable tile widths: a small first tile starts the store stream early,
# a small last tile minimizes the compute-drain after the final load.
WIDTHS = [1024] + [8192] * 11 + [1271] + [1024]
assert sum(WIDTHS) == C and max(WIDTHS) == T
NT = len(WIDTHS)              # 14

_CACHE: dict = {}
LAST_RESULT = None            # BassKernelResults of the last run (for test.py)
RUN_KWARGS: dict = {}         # test.py can set {"trace": True}


def _build():
    bf16 = mybir.dt.bfloat16
    # Bacc (not raw Bass): its compile pass splits multi-wait sync onto
    # separate event-semaphore instructions — DMACopy only encodes 1 wait.
    nc = bacc.Bacc(None, enable_partition_id=False)
    x = nc.declare_dram_parameter("x", [R, C], bf16, isOutput=False)
    y = nc.declare_dram_parameter("y", [R, C], bf16, isOutput=True)
    mx = nc.declare_dram_parameter("mx", [R, NT], bf16, isOutput=True)

    # 64*v is an exponent shift (exact in bf16), and {bf16 v: 64v <= 25.6f}
    # == {bf16 v: v <= 0.4f}, so filtering the scaled tensor matches the
    # reference filter on the bf16 grid bit-exactly.
    thresh_s = float(np.float32(THRESH) * np.float32(S))

    # Loads on the sync HWDGE ring, stores on the scalar engine's HWDGE
    # ring (same-engine ordering after the mul that produced the data).
    with tile.TileContext(nc) as tc:
        with (
            tc.tile_pool(name="xin", bufs=4) as xpool,
            tc.tile_pool(name="yout", bufs=4) as ypool,
            tc.tile_pool(name="gtmp", bufs=2) as gpool,
            tc.tile_pool(name="stat", bufs=1) as statpool,
        ):
            maxbuf = statpool.tile([R, NT], bf16)
            col = 0
            for t, w in enumerate(WIDTHS):
                xt = xpool.tile([R, T], bf16, tag="xt")
                nc.sync.dma_start(out=xt[:, :w], in_=x[:, col : col + w])

                yt = ypool.tile([R, T], bf16, tag="yt")
                nc.scalar.mul(yt[:, :w], xt[:, :w], S)
                nc.scalar.dma_start(out=y[:, col : col + w], in_=yt[:, :w])

                # g = (yt <= 25.6) * yt  (== 64*filtered(xb), exact bf16)
                g = gpool.tile([R, T], bf16, tag="g")
                nc.vector.scalar_tensor_tensor(
                    out=g[:, :w],
                    in0=yt[:, :w],
                    scalar=thresh_s,
                    in1=yt[:, :w],
                    op0=mybir.AluOpType.is_le,
                    op1=mybir.AluOpType.mult,
                )
                nc.vector.tensor_reduce(
                    out=maxbuf[:, t : t + 1],
                    in_=g[:, :w],
                    axis=mybir.AxisListType.X,
                    op=mybir.AluOpType.max,
                )
                col += w

            # ship the per-tile maxima; the final NT-column max runs on host
            nc.scalar.dma_start(out=mx[:], in_=maxbuf[:])
    # run_bass_via_pjrt serializes the module at jit-lowering time without
    # finalizing; Bacc's register allocation happens in finalize().
    nc.finalize()
    return nc


def _get_nc():
    if "nc" not in _CACHE:
        _CACHE["nc"] = _build()
    return _CACHE["nc"]


def kernel(logits, labels):
    global LAST_RESULT
    logits = np.ascontiguousarray(np.asarray(logits, dtype=np.float32))
    labels = np.asarray(labels).astype(np.int64)
    assert logits.shape == (N, C)

    # round-to-nearest f32 -> bf16; this IS the tensor the device sees
    xb16 = logits.astype(BF16)

    nc = _get_nc()
    in_maps = [{"x": xb16[k * R : (k + 1) * R]} for k in range(NCORES)]
    res = run_bass_kernel_spmd(nc, in_maps, list(range(NCORES)), **RUN_KWARGS)
    LAST_RESULT = res

    out = np.concatenate(
        [res.results[k]["y"] for k in range(NCORES)], axis=0
    ).astype(np.float32)                       # exact: 64 * f32(bf16(x))
    M64 = np.concatenate(
        [res.results[k]["mx"] for k in range(NCORES)], axis=0
    ).astype(np.float32).max(axis=1)           # exact bf16*64 grid values

    # ---- host glue: per-row scalars (N=1024) ----
    valid = labels != -1
    lab = np.where(valid, labels, 0)
    rows = np.arange(N)
    cos_y = logits[rows, lab]                                   # f32, exact
    thresh_s = np.float32(THRESH) * np.float32(S)

    # the device's (bf16-domain) view of the label column, scaled by 64
    cos_y_b64 = xb16[rows, lab].astype(np.float32) * np.float32(S)
    g_label64 = np.where(cos_y_b64 <= thresh_s, cos_y_b64, np.float32(0.0))

    max_other = (M64 * np.float32(1.0 / S)).astype(np.float32)  # exact scale

    h = (np.float32(1.0) - (cos_y - max_other)).astype(np.float32)
    m_i = (np.float32(M2) + np.float32(ALPHA) * h).astype(np.float32)
    theta = np.arccos(np.clip(cos_y, -1.0, 1.0)).astype(np.float32)
    phi = (np.cos(np.float32(M1) * theta + m_i) - np.float32(M3)).astype(np.float32)

    # rows needing the exact f32 recompute:
    #  - label column may dominate the device max (exact bf16-domain check)
    #  - |phi| small: bf16 max quantization (delta_phi <= ~4e-4) could
    #    exceed the 2e-2 relative gate at the label position
    risk = valid & ((g_label64 >= M64) | (np.abs(phi) < np.float32(0.1)))
    ridx = np.nonzero(risk)[0]
    if ridx.size:
        sub = logits[ridx]                                      # [K, C] f32
        gs = np.where(sub <= np.float32(THRESH), sub, np.float32(0.0))
        gs[np.arange(ridx.size), lab[ridx]] = np.float32(NEG_BIG)
        mo = gs.max(axis=1).astype(np.float32)
        max_other[ridx] = mo
        h_r = (np.float32(1.0) - (cos_y[ridx] - mo)).astype(np.float32)
        m_r = (np.float32(M2) + np.float32(ALPHA) * h_r).astype(np.float32)
        phi[ridx] = (
            np.cos(np.float32(M1) * theta[ridx] + m_r) - np.float32(M3)
        ).astype(np.float32)

    final_phi = np.where(phi < cos_y, phi, cos_y).astype(np.float32)
    out[rows[valid], lab[valid]] = final_phi[valid] * np.float32(S)
    return out
